# revision 21
# baseline (speedup 1.0000x reference)
"""Groupwise projection kernel for Trainium2 (8 NeuronCores).

Problem: x [16, 4096, 512] fp32; 8 contiguous token segments per 4096-token
row, each with its own Linear (W [8, 512, 512], b [8, 512]);
out[b, t, :] = x[b, t, :] @ W[g(t)].T + b[g(t)].

Strategy (v9, bf16 + startup/drain engineering):
  - All streams move as bf16 (x, W rounded host-side; out written bf16 by
    the PSUM->SBUF bias-add, upconverted host-side). Per-core HBM traffic
    8.39 (x) + 8.39 (out) + 1.57 (w) = 18.4MB. TensorE runs bf16 at
    1 cycle/row -> 131072 cycles = 54.6us at 2.4GHz; the kernel sits at
    the compute/memory ridge with PE as critical path. fp32 PSUM accum;
    measured rel err ~2.9e-3 (gate 2e-2).
  - Startup: the first matmul needs only the k0/k1 halves of slot-0
    weights and of the first 512-token x chunk. Those two half-loads lead
    the sync HWDGE queue (0.52MB -> land ~2us after DMA flow starts),
    followed by the k2/k3 halves, then the rest of the x stream in
    consumption order (one 512-token chunk, then 7 x 1024-token chunks).
    A single queue saturates the ~390 GB/s core DMA bandwidth; queue FIFO
    makes arrival order deterministic. Everything x stays resident in
    SBUF (64KB/partition), so nothing throttles the stream. Only the
    (tiny) bias rides the scalar queue at startup.
  - The Tile scheduler hoists dependency-free DMAs to the front, so the
    deferred w1/w2 loads each get an artificial input dep (a 1-element
    DVE write into their SBUF region reading tile-4/-9 output) pinning
    them mid-run on the gpsimd queue, clear of the startup burst but ~5
    tiles before first use.
  - PE p-state: the PE ramps 0.65->1.2->2.4GHz over ~3us of continuous
    work. Dummy matmuls on a memset tile (5x512-row + 6x128-row) keep the
    PE busy from the preamble until real data lands, so real matmuls run
    at (nearly) full clock from the first tile.
  - PSUM->SBUF bias-add copies alternate DVE (tensor_scalar_add) and ACT
    (activation Identity + bias) so neither engine rivals the PE.
  - Tokens are independent given their group: the host reshuffles tokens
    into 3 weight "slots" per core of (4096, 2560, 1536) tokens; the
    (core, slot) -> group map tiles the global work exactly, so each core
    loads only 3 of 8 weight matrices. All DRAM buffers are packed in
    exact DMA consumption order (fully sequential HBM streams).
  - Mid-run stores ride the gpsimd SWDGE queue; the last two tiles'
    stores move to the (by then idle) sync/scalar HWDGE queues, tile 15
    split per-ob so each piece leaves as soon as its copy finishes.
  - Host scatters the per-core outputs back into [16, 4096, 512] fp32.
"""

import sys

sys.path.insert(0, "/opt/trn_rl_repo")

import numpy as np
import ml_dtypes
import concourse.bacc as bacc
import concourse.bass as bass
import concourse.mybir as mybir
import concourse.tile as tile
from concourse.bass_utils import run_bass_kernel_spmd

# run_bass_kernel_spmd imports antenv.axon_hooks when BASS_TRACE is set; some
# images lack that module. Register a no-op fallback so a stray BASS_TRACE
# can only skip profiling, never crash the run.
try:
    import antenv.axon_hooks  # noqa: F401
except ImportError:
    import types

    _hooks = types.ModuleType("antenv.axon_hooks")
    _hooks._hook = None
    _hooks.set_axon_ntff_profile_hook = lambda h: setattr(_hooks, "_hook", h)
    _hooks.get_axon_ntff_profile_hook = lambda: _hooks._hook
    try:
        import antenv

        antenv.axon_hooks = _hooks
        sys.modules["antenv.axon_hooks"] = _hooks
    except ImportError:
        pass

F32 = mybir.dt.float32
BF16 = mybir.dt.bfloat16
NP_BF16 = ml_dtypes.bfloat16

LEN_GROUPS = (256, 512, 768, 384, 640, 512, 576, 448)
NUM_GROUPS, D_IN, D_OUT = 8, 512, 512
BATCH, T = 16, 4096
N_CORES = 8
T_CORE = 8192  # tokens per core (16*4096/8)
KB = D_IN // 128   # 4 contraction blocks
OB = D_OUT // 128  # 4 output blocks
NT = 512           # moving-dim tile (tokens per matmul)
N_TILES = T_CORE // NT

# Weight slots per core: slot s covers SLOT_SIZES[s] tokens, all of one group.
SLOT_SIZES = (4096, 2560, 1536)
N_SLOTS = 3
# (slot, core) -> group. Tiles the 16*L_g tokens of every group exactly.
SLOT_GROUPS = (
    (0, 1, 1, 2, 2, 2, 6, 7),  # 4096-token slots
    (4, 4, 4, 4, 5, 5, 6, 6),  # 2560-token slots
    (3, 3, 3, 3, 5, 5, 7, 7),  # 1536-token slots
)
# tile index -> slot index
TILE_SLOT = [0] * 8 + [1] * 5 + [2] * 3
# sync-queue FIFO: emit slot-s weight load just before this x chunk
W_LOAD_BEFORE_CHUNK = {4: 1, 6: 2}

# x chunks in tiles: two 1-tile chunks (fast start), then 2-tile chunks
CHUNK_TILES = [1, 1] + [2] * 7
CHUNK_START = np.concatenate([[0], np.cumsum(CHUNK_TILES)]).tolist()

_NC_CACHE = None
_LAST_RESULTS = None  # test harness introspection (exec_time_ns etc.)


def _token_assignment():
    """Per-core global token indices (into x.reshape(-1, 512)), slot-major."""
    starts = np.cumsum((0,) + LEN_GROUPS[:-1])
    pools = []
    for g in range(NUM_GROUPS):
        seg = np.arange(starts[g], starts[g] + LEN_GROUPS[g])
        pools.append(
            (np.arange(BATCH)[:, None] * T + seg[None, :]).reshape(-1)
        )
    used = [0] * NUM_GROUPS
    core_tok = [[] for _ in range(N_CORES)]
    for s in range(N_SLOTS):
        size = SLOT_SIZES[s]
        for c in range(N_CORES):
            g = SLOT_GROUPS[s][c]
            core_tok[c].append(pools[g][used[g]:used[g] + size])
            used[g] += size
    assert all(used[g] == BATCH * LEN_GROUPS[g] for g in range(NUM_GROUPS))
    return [np.concatenate(t) for t in core_tok]


TOKEN_INDEX = _token_assignment()


def _build_nc():
    nc = bacc.Bacc("TRN2", target_bir_lowering=False, debug=False,
                   num_devices=N_CORES)

    # All buffers packed in exact DMA consumption order (sequential HBM).
    xP = nc.dram_tensor("xP", [D_IN * T_CORE], BF16, kind="ExternalInput").ap()
    wP = nc.dram_tensor("wP", [N_SLOTS * D_IN * D_OUT], BF16,
                        kind="ExternalInput").ap()
    bS = nc.dram_tensor("bS", [128, N_SLOTS * OB], F32,
                        kind="ExternalInput").ap()
    oP = nc.dram_tensor("oP", [D_OUT * T_CORE], BF16, kind="ExternalOutput").ap()

    w_len = D_IN * D_OUT

    def w_dram(s, k0=0, k1=KB):
        # [s][p][k][o] packing; k-slice keeps per-partition runs contiguous
        return (
            wP[s * w_len:(s + 1) * w_len]
            .rearrange("(p k o) -> p k o", p=128, k=KB)[:, k0:k1, :]
        )

    def x_dram(t0, t1, k0=0, k1=KB):
        # [p][k][t] packing per chunk [t0, t1) in tiles
        return (
            xP[t0 * NT * D_IN:t1 * NT * D_IN]
            .rearrange("(p k t) -> p k t", p=128, k=KB)[:, k0:k1, :]
        )

    with tile.TileContext(nc) as tc:
        with (
            tc.tile_pool(name="wpool", bufs=1) as wpool,
            tc.tile_pool(name="bpool", bufs=1) as bpool,
            tc.tile_pool(name="warm", bufs=1) as warmpool,
            tc.tile_pool(name="xpool", bufs=len(CHUNK_TILES)) as xpool,
            tc.tile_pool(name="opool", bufs=4) as opool,
            tc.tile_pool(name="psum", bufs=6, space=bass.MemorySpace.PSUM) as psum,
            tc.tile_pool(name="wpsum", bufs=1, space=bass.MemorySpace.PSUM) as wpsum,
        ):
            # Weights resident in SBUF: [p, s, k, o] = W^T[g_s][k*128+p, o]
            w_sb = wpool.tile([128, N_SLOTS, KB, D_OUT], BF16)
            b_sb = bpool.tile([128, N_SLOTS * OB], F32)
            warm_sb = warmpool.tile([128, NT], BF16)

            # The first matmuls need only the k0/k1 halves of slot-0 weights
            # and of the first 512-token x chunk. The two k01 halves lead
            # the sync and scalar queues (parallel trigger issue, ~0.26MB
            # each -> land ~1.5us after DMA flow starts), the k23 halves
            # follow, then the whole x stream rides the sync queue FIFO in
            # consumption order with the deferred w1/w2 loads interleaved
            # at their consumption points.
            x_sb = []
            for ci, ntile in enumerate(CHUNK_TILES):
                xt = xpool.tile([128, KB, ntile * NT], BF16, tag="x")
                x_sb.append(xt)
            h = KB // 2
            nc.sync.dma_start(w_sb[:, 0, 0:h, :], w_dram(0, 0, h))
            nc.sync.dma_start(x_sb[0][:, 0:h, :], x_dram(0, 1, 0, h))
            nc.sync.dma_start(w_sb[:, 0, h:KB, :], w_dram(0, h, KB))
            nc.sync.dma_start(x_sb[0][:, h:KB, :], x_dram(0, 1, h, KB))
            nc.scalar.dma_start(b_sb[:], bS)
            for ci in range(1, len(CHUNK_TILES)):
                if ci in W_LOAD_BEFORE_CHUNK:
                    ws = W_LOAD_BEFORE_CHUNK[ci]
                    nc.sync.dma_start(w_sb[:, ws, :, :], w_dram(ws))
                nc.sync.dma_start(
                    x_sb[ci][:],
                    x_dram(CHUNK_START[ci], CHUNK_START[ci + 1]),
                )

            # PE p-state warmup: keep the array busy from the preamble until
            # the first chunk lands so real matmuls start at full clock.
            nc.vector.memset(warm_sb[:], 0)
            warm_ps = wpsum.tile([128, NT], F32)
            for _ in range(5):
                nc.tensor.matmul(warm_ps[:], warm_sb[:, 0:128], warm_sb[:],
                                 start=True, stop=True)
            for _ in range(18):
                nc.tensor.matmul(warm_ps[:, 0:128], warm_sb[:, 0:128],
                                 warm_sb[:, 0:128], start=True, stop=True)

            for i in range(N_TILES):  # 16 tiles of 512 tokens
                s = TILE_SLOT[i]
                ci = next(j for j in range(len(CHUNK_TILES))
                          if CHUNK_START[j] <= i < CHUNK_START[j + 1])
                co = (i - CHUNK_START[ci]) * NT
                o_sb = opool.tile([128, OB, NT], BF16, tag="o")
                for ob in range(OB):
                    acc = psum.tile([128, NT], F32, tag="acc")
                    for k in range(KB):
                        nc.tensor.matmul(
                            acc[:],
                            w_sb[:, s, k, ob * 128:(ob + 1) * 128],
                            x_sb[ci][:, k, co:co + NT],
                            start=(k == 0),
                            stop=(k == KB - 1),
                        )
                    # PSUM -> SBUF with bias, alternating DVE / ACT so neither
                    # engine rivals the PE (fp32 acc + fp32 bias -> bf16 out)
                    bias_ap = b_sb[:, s * OB + ob:s * OB + ob + 1]
                    if ob % 2 == 0:
                        nc.vector.tensor_scalar_add(o_sb[:, ob, :], acc[:],
                                                    bias_ap)
                    else:
                        nc.scalar.add(o_sb[:, ob, :], acc[:], bias_ap)
                tile_len = 128 * OB * NT
                o_dram = oP[i * tile_len:(i + 1) * tile_len].rearrange(
                    "(p ob t) -> p ob t", p=128, ob=OB
                )
                if i == N_TILES - 1:
                    # per-ob, alternating engines, so each piece leaves as
                    # soon as its copy is done with parallel trigger issue
                    for ob in range(OB):
                        eng = nc.sync if ob % 2 == 0 else nc.scalar
                        eng.dma_start(o_dram[:, ob:ob + 1, :],
                                      o_sb[:, ob:ob + 1, :])
                else:
                    # mid-run stores ride the scalar HWDGE queue (compute-
                    # paced), keeping them off the x queue; the gpsimd/SWDGE
                    # path stays completely unused (cheaper teardown)
                    nc.scalar.dma_start(o_dram, o_sb[:])

    nc.compile()
    return nc


def kernel(x: np.ndarray, W: np.ndarray, b: np.ndarray) -> np.ndarray:
    global _NC_CACHE, _LAST_RESULTS
    x = np.asarray(x, dtype=np.float32)
    W = np.asarray(W, dtype=np.float32)
    b = np.asarray(b, dtype=np.float32)

    if _NC_CACHE is None:
        _NC_CACHE = _build_nc()
    nc = _NC_CACHE

    wT = np.ascontiguousarray(W.transpose(0, 2, 1)).astype(NP_BF16)  # [g,d,o]
    x_bf = x.reshape(BATCH * T, D_IN).astype(NP_BF16)

    in_maps = []
    for c in range(N_CORES):
        groups = [SLOT_GROUPS[s][c] for s in range(N_SLOTS)]
        # wP packed [s][p][k][o] = wT[g_s][k*128+p, o]
        wsel = wT[groups]  # [3, 512, 512] = [s, (k p), o]
        wP = np.ascontiguousarray(
            wsel.reshape(N_SLOTS, KB, 128, D_OUT).transpose(0, 2, 1, 3)
        ).reshape(-1)
        # bias laid out [p, s*4 + ob] = b[g_s, ob*128 + p]
        bS = np.ascontiguousarray(
            b[groups].reshape(N_SLOTS, OB, 128).transpose(2, 0, 1)
            .reshape(128, N_SLOTS * OB)
        )
        # xP packed per chunk as [p][k][t]:
        # (chunk, p, k, t) = x^T[k*128+p, chunk_start*512+t]
        xc = x_bf[TOKEN_INDEX[c]]  # [8192, 512] bf16
        parts = []
        for ci, ntile in enumerate(CHUNK_TILES):
            t0, t1 = CHUNK_START[ci] * NT, CHUNK_START[ci + 1] * NT
            blk = xc[t0:t1]  # [ct, 512] = [t, (k p)]
            parts.append(
                np.ascontiguousarray(
                    blk.reshape(t1 - t0, KB, 128).transpose(2, 1, 0)
                ).reshape(-1)
            )
        xP = np.concatenate(parts)
        in_maps.append({"xP": xP, "wP": wP, "bS": bS})

    res = run_bass_kernel_spmd(nc, in_maps, list(range(N_CORES)))
    _LAST_RESULTS = res

    out = np.empty((BATCH * T, D_OUT), dtype=np.float32)
    for c in range(N_CORES):
        oc = np.asarray(res.results[c]["oP"]).reshape(N_TILES, 128, OB, NT)
        # [tile, p, ob, t] -> [tile, t, (ob p) = o]
        oc = oc.transpose(0, 3, 2, 1).reshape(T_CORE, D_OUT).astype(np.float32)
        out[TOKEN_INDEX[c]] = oc
    return out.reshape(BATCH, T, D_OUT)
